# revision 9
# baseline (speedup 1.0000x reference)
"""BUrumorGAT on 8 TRN2 NeuronCores via Bass/Tile.

Strategy (3 SPMD launches, host exchanges between them are free for HW time):
  L1: per-core node-sharded  g1 = [x@W1 | x@Vs1 | x@Vd1]  (attention vectors
      folded into the weight matrix on host).
  host: assemble full g1 table; expand per-graph term gt = relu(x[root])@W2extB.
  L2: per-core dst-sharded conv1 aggregation via dma_gather + one-hot matmul;
      epilogue computes h1, then g2 = relu(h1)@W2extA + gt.
  host: assemble full g2 table.
  L3: conv2 aggregation the same way; h2 relu'd and pooled per-graph via
      one-hot matmul into a persistent PSUM bank.
  host: combine partial pooled sums, add counts*h1[root] (re2 part), divide.

Edges use dst-sorted order, blocks of 128 dsts, gathers split into
src<32768 / src>=32768 halves (int16 index limit of dma_gather).
"""
import sys, os, math
sys.path.insert(0, "/opt/trn_rl_repo")
import numpy as np
from contextlib import ExitStack

import concourse.bass as bass
import concourse.mybir as mybir
import concourse.tile as tile
from concourse import bacc
from concourse.bass_utils import run_bass_kernel_spmd

F32 = mybir.dt.float32
I16 = mybir.dt.int16
P = 128
NCORES = 8
NEG = 0.2
EPS = 1e-16
I16_LIM = 32768

# problem constants (hardcoded per contract)
N, E0, B = 50000, 800000, 512
IN, HID, OUT, HEADS = 768, 256, 128, 4
D1, D2 = HID // HEADS, OUT // HEADS
G1W = 320          # g1 row width f32 (1280B): [hpre1 256 | a_s1 4 | a_d1 4 | pad]
G2W = 192          # g2 row width f32 (768B):  [hpre2 128 | a_s2 4 | a_d2 4 | pad]

_trace = bool(int(os.environ.get("KERNEL_TRACE", "0")))
_cache = {}


def _wrap16(a):
    # dma_gather idx layout: index i at [i % 16, i // 16], tiled over 8 q7 cores
    return np.tile(a.reshape(-1, 16).T, (8, 1)).astype(np.int16)


MAXT = 8  # dma_gather supports at most 1024 indices (8 e-tiles) per call


def _chunks(nt):
    return [(s, min(MAXT, nt - s)) for s in range(0, nt, MAXT)]


def _wrap_chunks(flat, nt):
    # per-call wrapping: each <=8-tile chunk is wrapped independently
    cols = []
    for s, c in _chunks(nt):
        cols.append(_wrap16(flat[s * P:(s + c) * P]))
    return np.concatenate(cols, axis=1)


# --------------------------------------------------------------------------
# L1: g1 rows for local nodes
# --------------------------------------------------------------------------
def build_l1(nb, kin, wext_w):
    """nb 128-node blocks; kin = IN//128 k-chunks; wext width (264)."""
    nc = bacc.Bacc("TRN2", target_bir_lowering=False, debug=False, num_devices=NCORES)
    xt = nc.dram_tensor("xt", [nb, P, kin, P], F32, kind="ExternalInput").ap()
    wext = nc.dram_tensor("wext", [P, kin, wext_w], F32, kind="ExternalInput").ap()
    g1o = nc.dram_tensor("g1o", [nb * P, G1W], F32, kind="ExternalOutput").ap()

    with tile.TileContext(nc) as tc, ExitStack() as ctx:
        cpool = ctx.enter_context(tc.tile_pool(name="consts", bufs=1))
        sbuf = ctx.enter_context(tc.tile_pool(name="sbuf", bufs=3))
        psum = ctx.enter_context(tc.tile_pool(name="psum", bufs=2, space="PSUM"))
        wt = cpool.tile([P, kin, wext_w], F32)
        nc.sync.dma_start(out=wt[:], in_=wext)
        for b in range(nb):
            xtt = sbuf.tile([P, kin, P], F32, tag="xt")
            nc.sync.dma_start(out=xtt[:], in_=xt[b])
            ps = psum.tile([P, wext_w], F32, space="PSUM", tag="ps")
            for k in range(kin):
                nc.tensor.matmul(out=ps[:], lhsT=xtt[:, k, :], rhs=wt[:, k, :],
                                 start=(k == 0), stop=(k == kin - 1))
            ot = sbuf.tile([P, wext_w], F32, tag="ot")
            nc.vector.tensor_copy(out=ot[:], in_=ps[:])
            nc.sync.dma_start(out=g1o[b * P:(b + 1) * P, 0:wext_w], in_=ot[:])
    nc.finalize()
    return nc


# --------------------------------------------------------------------------
# L2 / L3 edge-aggregation launch builder
# --------------------------------------------------------------------------
def build_edge(nb, tlow, thigh, gtab_rows, gw, msgw, heads, hd, shard_rows,
               final_pool):
    """One aggregation layer.

    nb: dst blocks per core; tlow/thigh: low/high e-tiles per block;
    gtab_rows: rows of the full gather table; gw: table row width (f32);
    msgw: message width (256 or 128); heads*hd == msgw.
    shard_rows: valid rows of this core's dst shard (for the a_d gather).
    final_pool: False -> L2 epilogue (h1, g2 out); True -> L3 (relu+pool).
    """
    T = tlow + thigh
    WI = (tlow + thigh + T) * 8          # idx tile width (int16 cols)
    aw = msgw + 4                        # aggregated width: msg + denom
    nc = bacc.Bacc("TRN2", target_bir_lowering=False, debug=False, num_devices=NCORES)

    gtab = nc.dram_tensor("gtab", [gtab_rows, gw], F32, kind="ExternalInput").ap()
    adtab = nc.dram_tensor("adtab", [shard_rows, gw], F32, kind="ExternalInput").ap()
    idx = nc.dram_tensor("idx", [nb, P, WI], I16, kind="ExternalInput").ap()
    dstloc = nc.dram_tensor("dstloc", [nb, P, T], F32, kind="ExternalInput").ap()
    iota = nc.dram_tensor("iota", [P, P], F32, kind="ExternalInput").ap()
    if not final_pool:
        gt_in = nc.dram_tensor("gt", [nb * P, HID // 2 + 8], F32, kind="ExternalInput").ap()
        ident = nc.dram_tensor("ident", [P, P], F32, kind="ExternalInput").ap()
        b1b = nc.dram_tensor("b1b", [P, HID], F32, kind="ExternalInput").ap()
        w2a = nc.dram_tensor("w2a", [P, HID // P, HID // 2 + 8], F32, kind="ExternalInput").ap()
        h1o = nc.dram_tensor("h1o", [nb * P, HID], F32, kind="ExternalOutput").ap()
        g2o = nc.dram_tensor("g2o", [nb * P, G2W], F32, kind="ExternalOutput").ap()
    else:
        b2b = nc.dram_tensor("b2b", [P, OUT], F32, kind="ExternalInput").ap()
        bl = nc.dram_tensor("bl", [P, nb], F32, kind="ExternalInput").ap()
        poolo = nc.dram_tensor("poolo", [P, OUT], F32, kind="ExternalOutput").ap()

    with tile.TileContext(nc) as tc, ExitStack() as ctx:
        cpool = ctx.enter_context(tc.tile_pool(name="consts", bufs=1))
        sbuf = ctx.enter_context(tc.tile_pool(name="sbuf", bufs=4))
        psum = ctx.enter_context(tc.tile_pool(name="psum", bufs=2, space="PSUM"))
        ppers = ctx.enter_context(tc.tile_pool(name="ppers", bufs=1, space="PSUM"))

        iota_t = cpool.tile([P, P], F32)
        nc.sync.dma_start(out=iota_t[:], in_=iota)
        if not final_pool:
            ident_t = cpool.tile([P, P], F32)
            nc.sync.dma_start(out=ident_t[:], in_=ident)
            b1t = cpool.tile([P, HID], F32)
            nc.sync.dma_start(out=b1t[:], in_=b1b)
            w2at = cpool.tile([P, HID // P, HID // 2 + 8], F32)
            nc.sync.dma_start(out=w2at[:], in_=w2a)
        else:
            b2t = cpool.tile([P, OUT], F32)
            nc.sync.dma_start(out=b2t[:], in_=b2b)
            blt = cpool.tile([P, nb], F32)
            nc.sync.dma_start(out=blt[:], in_=bl)
            pool_ps = ppers.tile([P, OUT], F32, space="PSUM")

        for b in range(nb):
            idxt = sbuf.tile([P, WI], I16, tag="idx")
            nc.sync.dma_start(out=idxt[:], in_=idx[b])
            dlt = sbuf.tile([P, T], F32, tag="dl")
            nc.sync.dma_start(out=dlt[:], in_=dstloc[b])

            G = sbuf.tile([P, T, gw], F32, tag="G")
            for s, cchunk in _chunks(tlow):
                nc.gpsimd.dma_gather(
                    out_ap=G[:, s:s + cchunk, :], in_ap=gtab,
                    idxs_ap=idxt[:, s * 8:(s + cchunk) * 8],
                    num_idxs=cchunk * P, num_idxs_reg=cchunk * P, elem_size=gw)
            for s, cchunk in _chunks(thigh):
                nc.gpsimd.dma_gather(
                    out_ap=G[:, tlow + s:tlow + s + cchunk, :], in_ap=gtab[I16_LIM:, :],
                    idxs_ap=idxt[:, (tlow + s) * 8:(tlow + s + cchunk) * 8],
                    num_idxs=cchunk * P, num_idxs_reg=cchunk * P, elem_size=gw)
            AD = sbuf.tile([P, T, 64], F32, tag="AD")
            for s, cchunk in _chunks(T):
                nc.gpsimd.dma_gather(
                    out_ap=AD[:, s:s + cchunk, :], in_ap=adtab[:, msgw:msgw + 64],
                    idxs_ap=idxt[:, (T + s) * 8:(T + s + cchunk) * 8],
                    num_idxs=cchunk * P, num_idxs_reg=cchunk * P,
                    elem_size=64, elem_step=gw)

            # z = a_s[src] + a_d[dst]; e = lrelu(z); expe = exp(e)
            Z = sbuf.tile([P, T, 4], F32, tag="Z")
            nc.vector.tensor_tensor(out=Z[:], in0=G[:, :, msgw:msgw + 4],
                                    in1=AD[:, :, 4:8], op=mybir.AluOpType.add)
            Z2 = sbuf.tile([P, T, 4], F32, tag="Z2")
            nc.vector.tensor_scalar_mul(out=Z2[:], in0=Z[:], scalar1=NEG)
            nc.vector.tensor_tensor(out=Z2[:], in0=Z[:], in1=Z2[:], op=mybir.AluOpType.max)
            EX = sbuf.tile([P, T, 4], F32, tag="EX")
            nc.scalar.activation(out=EX[:], in_=Z2[:], func=mybir.ActivationFunctionType.Exp)

            # weight messages in place; denom columns get expe
            exb = EX[:].to_broadcast([P, T, heads, hd])
            gv = G[:, :, 0:msgw].rearrange("p t (h j) -> p t h j", h=heads)
            nc.vector.tensor_tensor(out=gv, in0=gv, in1=exb, op=mybir.AluOpType.mult)
            nc.vector.tensor_copy(out=G[:, :, msgw:msgw + 4], in_=EX[:])

            agg = psum.tile([P, aw], F32, space="PSUM", tag="agg")
            # all T one-hot matrices in one DVE op: S[p,t,d] = (iota[p,d]==dl[p,t])
            Sall = sbuf.tile([P, T, P], F32, tag="S")
            nc.vector.tensor_tensor(
                out=Sall[:],
                in0=iota_t[:, None, :].to_broadcast([P, T, P]),
                in1=dlt[:, :, None].to_broadcast([P, T, P]),
                op=mybir.AluOpType.is_equal)
            for t in range(T):
                nc.tensor.matmul(out=agg[:], lhsT=Sall[:, t, :], rhs=G[:, t, 0:aw],
                                 start=(t == 0), stop=(t == T - 1))

            # epilogue: h = agg[:, :msgw] * recip(denom + eps)  (+ bias)
            rcp = sbuf.tile([P, 4], F32, tag="rcp")
            nc.vector.tensor_scalar_add(out=rcp[:], in0=agg[:, msgw:msgw + 4], scalar1=EPS)
            nc.vector.reciprocal(out=rcp[:], in_=rcp[:])
            ht = sbuf.tile([P, msgw], F32, tag="ht")
            hv = ht[:].rearrange("p (h j) -> p h j", h=heads)
            av = agg[:, 0:msgw].rearrange("p (h j) -> p h j", h=heads)
            nc.vector.tensor_tensor(out=hv, in0=av,
                                    in1=rcp[:].to_broadcast([P, heads, hd]),
                                    op=mybir.AluOpType.mult)

            if not final_pool:
                nc.vector.tensor_tensor(out=ht[:], in0=ht[:], in1=b1t[:],
                                        op=mybir.AluOpType.add)
                nc.sync.dma_start(out=h1o[b * P:(b + 1) * P, :], in_=ht[:])
                r1 = sbuf.tile([P, msgw], F32, tag="r1")
                nc.vector.tensor_scalar_max(out=r1[:], in0=ht[:], scalar1=0.0)
                # transpose r1 (msgw//P chunks) then project through W2extA
                kk = msgw // P
                trp = psum.tile([P, kk, P], F32, space="PSUM", tag="trp")
                for c in range(kk):
                    nc.tensor.transpose(out=trp[:, c, :], in_=r1[:, c * P:(c + 1) * P],
                                        identity=ident_t[:])
                r1T = sbuf.tile([P, kk, P], F32, tag="r1T")
                nc.vector.tensor_copy(out=r1T[:], in_=trp[:])
                ps2 = psum.tile([P, HID // 2 + 8], F32, space="PSUM", tag="ps2")
                for c in range(kk):
                    nc.tensor.matmul(out=ps2[:], lhsT=r1T[:, c, :], rhs=w2at[:, c, :],
                                     start=(c == 0), stop=(c == kk - 1))
                gtt = sbuf.tile([P, HID // 2 + 8], F32, tag="gtt")
                nc.sync.dma_start(out=gtt[:], in_=gt_in[b * P:(b + 1) * P, :])
                g2t = sbuf.tile([P, HID // 2 + 8], F32, tag="g2t")
                nc.vector.tensor_tensor(out=g2t[:], in0=ps2[:], in1=gtt[:],
                                        op=mybir.AluOpType.add)
                nc.sync.dma_start(out=g2o[b * P:(b + 1) * P, 0:HID // 2 + 8], in_=g2t[:])
            else:
                nc.vector.tensor_tensor(out=ht[:], in0=ht[:], in1=b2t[:],
                                        op=mybir.AluOpType.add)
                nc.vector.tensor_scalar_max(out=ht[:], in0=ht[:], scalar1=0.0)
                Bg = sbuf.tile([P, P], F32, tag="Bg")
                nc.vector.tensor_tensor(
                    out=Bg[:], in0=iota_t[:],
                    in1=blt[:, b:b + 1].to_broadcast([P, P]),
                    op=mybir.AluOpType.is_equal)
                nc.tensor.matmul(out=pool_ps[:], lhsT=Bg[:], rhs=ht[:],
                                 start=(b == 0), stop=(b == nb - 1))

        if final_pool:
            po = cpool.tile([P, OUT], F32)
            nc.vector.tensor_copy(out=po[:], in_=pool_ps[:])
            nc.sync.dma_start(out=poolo, in_=po[:])
    nc.finalize()
    return nc


# --------------------------------------------------------------------------
# host orchestration
# --------------------------------------------------------------------------
def _run(nc, in_maps, label, out_names=()):
    if os.environ.get("KERNEL_DUMP") == label:
        np.savez(f"/tmp/{label}_in.npz", **in_maps[0])
        print(f"dumped {label} inputs")
    if os.environ.get("KERNEL_SIM"):
        from concourse.bass_interp import CoreSim
        outs = []
        for c, im in enumerate(in_maps):
            sim = CoreSim(nc, trace=False)
            for k, v in im.items():
                sim.tensor(k)[:] = v
            sim.simulate(check_with_hw=False)
            outs.append({k: np.array(sim.tensor(k)) for k in out_names})
            print(f"[{label}] sim core {c} done")
        return outs
    res = run_bass_kernel_spmd(nc, in_maps, core_ids=list(range(NCORES)),
                               trace=_trace)
    if _trace and res.exec_time_ns:
        print(f"[{label}] HW exec time: {res.exec_time_ns} ns")
    _cache.setdefault("times", {})[label] = res.exec_time_ns
    return res.results


def kernel(x, edge_index, batch, W1, att_src1, att_dst1, b1,
           W2, att_src2, att_dst2, b2):
    x = np.asarray(x, np.float32)
    edge_index = np.asarray(edge_index)
    batch = np.asarray(batch).astype(np.int64)
    W1 = np.asarray(W1, np.float32); W2 = np.asarray(W2, np.float32)
    att_src1 = np.asarray(att_src1, np.float32); att_dst1 = np.asarray(att_dst1, np.float32)
    att_src2 = np.asarray(att_src2, np.float32); att_dst2 = np.asarray(att_dst2, np.float32)
    b1 = np.asarray(b1, np.float32); b2 = np.asarray(b2, np.float32)

    SH = N // NCORES                     # 6250 dst rows per core
    NB = math.ceil(SH / P)               # 49 blocks
    SHP = NB * P                         # 6272 padded
    KIN = IN // P

    # ---- edges: swap direction, add self-loops, sort by dst ----
    src = np.concatenate([edge_index[1], np.arange(N)]).astype(np.int64)
    dst = np.concatenate([edge_index[0], np.arange(N)]).astype(np.int64)
    order = np.argsort(dst, kind="stable")
    src, dst = src[order], dst[order]

    # ---- per-block low/high edge lists ----
    # blocks are shard-relative: core c, local block lb covers dst rows
    # [c*SH + lb*128, c*SH + (lb+1)*128) clipped to the shard
    blk = (dst // SH) * NB + (dst % SH) // P
    nblk = NB * NCORES
    bnd = np.searchsorted(blk, np.arange(nblk + 1))
    is_low = src < I16_LIM
    # per-block counts
    tlow_need = thigh_need = 0
    for gb in range(nblk):
        s, e = bnd[gb], bnd[gb + 1]
        nl = int(is_low[s:e].sum()); nh = (e - s) - nl
        tlow_need = max(tlow_need, nl); thigh_need = max(thigh_need, nh)
    TLOW = math.ceil(tlow_need / P)
    THIGH = math.ceil(max(thigh_need, 1) / P)
    T = TLOW + THIGH
    WI = (TLOW + THIGH + T) * 8

    idx_arr = np.zeros((NCORES, NB, P, WI), np.int16)
    dl_arr = np.full((NCORES, NB, P, T), 999.0, np.float32)
    for gb in range(nblk):
        c, lb = divmod(gb, NB)
        s, e = bnd[gb], bnd[gb + 1]
        es, ed = src[s:e], dst[s:e]
        ml = es < I16_LIM
        low_s, low_d = es[ml], ed[ml]
        high_s, high_d = es[~ml], ed[~ml]
        nl, nh = len(low_s), len(high_s)
        lowp = np.zeros(TLOW * P, np.int64); lowp[:nl] = low_s
        highp = np.zeros(THIGH * P, np.int64); highp[:nh] = high_s - I16_LIM
        # dst-local values in tile order (tile t edge i=t*128+p)
        dloc = np.full(T * P, 999.0, np.float32)
        dloc[:nl] = (low_d - c * SH) % P
        dloc[TLOW * P:TLOW * P + nh] = (high_d - c * SH) % P
        # a_d gather: local shard row of dst
        adl = np.zeros(T * P, np.int64)
        adl[:nl] = low_d - c * SH
        adl[TLOW * P:TLOW * P + nh] = high_d - c * SH
        idx_arr[c, lb, :, 0:TLOW * 8] = _wrap_chunks(lowp, TLOW)
        idx_arr[c, lb, :, TLOW * 8:(TLOW + THIGH) * 8] = _wrap_chunks(highp, THIGH)
        idx_arr[c, lb, :, (TLOW + THIGH) * 8:] = _wrap_chunks(adl, T)
        dl_arr[c, lb] = dloc.reshape(T, P).T

    # ---- graphs / roots / counts ----
    roots = np.searchsorted(batch, np.arange(B), "left")   # first node of graph
    counts = np.bincount(batch, minlength=B).astype(np.float32)
    gstart = batch[np.arange(NCORES) * SH]                 # first graph of core
    bl_arr = np.full((NCORES, NB * P), 999.0, np.float32)
    for c in range(NCORES):
        loc = batch[c * SH:(c + 1) * SH] - gstart[c]
        assert loc.max() < P, "graphs per core exceed 128"
        bl_arr[c, :SH] = loc
    bl_tile = bl_arr.reshape(NCORES, NB, P).transpose(0, 2, 1).copy()  # [c, P, NB]

    # ---- folded weights ----
    Vs1 = np.concatenate([W1[:, h * D1:(h + 1) * D1] @ att_src1[h] for h in range(HEADS)]).reshape(HEADS, IN).T
    Vd1 = np.concatenate([W1[:, h * D1:(h + 1) * D1] @ att_dst1[h] for h in range(HEADS)]).reshape(HEADS, IN).T
    W1ext = np.concatenate([W1, Vs1, Vd1], axis=1).astype(np.float32)      # [768, 264]
    Vs2 = np.concatenate([W2[:, h * D2:(h + 1) * D2] @ att_src2[h] for h in range(HEADS)]).reshape(HEADS, HID + IN).T
    Vd2 = np.concatenate([W2[:, h * D2:(h + 1) * D2] @ att_dst2[h] for h in range(HEADS)]).reshape(HEADS, HID + IN).T
    W2ext = np.concatenate([W2, Vs2, Vd2], axis=1).astype(np.float32)      # [1024, 136]
    W2extA, W2extB = W2ext[:HID], W2ext[HID:]

    iota_h = np.ascontiguousarray(np.broadcast_to(np.arange(P, dtype=np.float32), (P, P)))
    ident_h = np.eye(P, dtype=np.float32)
    b1b_h = np.ascontiguousarray(np.broadcast_to(b1, (P, HID))).astype(np.float32)
    b2b_h = np.ascontiguousarray(np.broadcast_to(b2, (P, OUT))).astype(np.float32)
    w1ext_pre = np.ascontiguousarray(W1ext.reshape(KIN, P, 264).transpose(1, 0, 2))
    w2a_pre = np.ascontiguousarray(W2extA.reshape(HID // P, P, 136).transpose(1, 0, 2))

    # ---- L1 ----
    l1 = _cache.get("l1")
    if l1 is None:
        l1 = _cache["l1"] = build_l1(NB, KIN, 264)
    xpad = np.zeros((NCORES, SHP, IN), np.float32)
    for c in range(NCORES):
        xpad[c, :SH] = x[c * SH:(c + 1) * SH]
    xt_pre = xpad.reshape(NCORES, NB, P, KIN, P).transpose(0, 1, 4, 3, 2)
    xt_pre = np.ascontiguousarray(xt_pre)
    in1 = [{"xt": xt_pre[c], "wext": w1ext_pre} for c in range(NCORES)]
    r1 = _run(l1, in1, "L1", ("g1o",))
    g1 = np.zeros((N, G1W), np.float32)
    for c in range(NCORES):
        g1[c * SH:(c + 1) * SH, 0:264] = r1[c]["g1o"][:SH, 0:264]

    # ---- host: graph term ----
    gt_graph = np.maximum(x[roots], 0.0) @ W2extB          # [B, 136]
    gt_node = np.zeros((NCORES, SHP, 136), np.float32)
    for c in range(NCORES):
        gt_node[c, :SH] = gt_graph[batch[c * SH:(c + 1) * SH]]

    # ---- L2 ----
    l2 = _cache.get("l2")
    if l2 is None:
        l2 = _cache["l2"] = build_edge(NB, TLOW, THIGH, N, G1W, HID, HEADS, D1, SH, False)
    _cache["edge_params"] = (NB, TLOW, THIGH, N, G1W, HID, HEADS, D1, SH)
    in2 = [{"gtab": g1, "adtab": np.ascontiguousarray(g1[c * SH:(c + 1) * SH]), "idx": idx_arr[c],
            "dstloc": dl_arr[c], "iota": iota_h, "gt": gt_node[c],
            "ident": ident_h, "b1b": b1b_h, "w2a": w2a_pre}
           for c in range(NCORES)]
    r2 = _run(l2, in2, "L2", ("h1o", "g2o"))
    g2 = np.zeros((N, G2W), np.float32)
    h1 = np.zeros((N, HID), np.float32)
    for c in range(NCORES):
        g2[c * SH:(c + 1) * SH, 0:136] = r2[c]["g2o"][:SH, 0:136]
        h1[c * SH:(c + 1) * SH] = r2[c]["h1o"][:SH]

    # ---- L3 ----
    l3 = _cache.get("l3")
    if l3 is None:
        l3 = _cache["l3"] = build_edge(NB, TLOW, THIGH, N, G2W, OUT, HEADS, D2, SH, True)
    in3 = [{"gtab": g2, "adtab": np.ascontiguousarray(g2[c * SH:(c + 1) * SH]), "idx": idx_arr[c],
            "dstloc": dl_arr[c], "iota": iota_h, "b2b": b2b_h, "bl": bl_tile[c]}
           for c in range(NCORES)]
    r3 = _run(l3, in3, "L3", ("poolo",))

    pooled_h2 = np.zeros((B, OUT), np.float32)
    for c in range(NCORES):
        ng = int(batch[(c + 1) * SH - 1] - gstart[c] + 1)
        pooled_h2[gstart[c]:gstart[c] + ng] += r3[c]["poolo"][:ng]

    pooled_re2 = counts[:, None] * h1[np.clip(roots, 0, N - 1)]
    out = np.concatenate([pooled_h2, pooled_re2], axis=1) / np.clip(counts, 1.0, None)[:, None]
    return out.astype(np.float32)
